# revision 1
# baseline (speedup 1.0000x reference)
"""Causal self-attention on 8 NeuronCores (TRN2), tensor-parallel over heads.

Reference: y = proj(softmax(causal(Q K^T / sqrt(64))) V) with
B=4, T=2048, D=1024, H=16 heads, head_dim=64.

Sharding: each core owns 2 heads (a 128-column slice of the Q/K/V
projections and the matching 128 rows of w_proj) for all batches. Each
core emits a partial [B*T, D] output; the host sums the 8 partials
(row-parallel matmul unshard) and reshapes to [B, T, D].
"""

import sys

for _p in ("/opt/trn_rl_repo",):
    if _p not in sys.path:
        sys.path.insert(0, _p)

import numpy as np

import concourse.bass as bass
import concourse.bacc as bacc
import concourse.mybir as mybir
from concourse import tile
from concourse.bass_utils import run_bass_kernel_spmd
from concourse.masks import make_identity

B, T, D, H = 4, 2048, 1024, 16
HD = D // H           # 64 head dim
NCORES = 8
HPC = H // NCORES     # 2 heads per core
CW = HPC * HD         # 128: per-core qkv column slice width
BT = B * T            # 8192 tokens
KC = D // 128         # 8 contraction chunks for the qkv projection
NQ = 512              # query chunk
NG = NQ // 128        # 4 key-tiles per S^T group
F32 = mybir.dt.float32
F32R = mybir.dt.float32r
EXP = mybir.ActivationFunctionType.Exp

VST = HPC * (HD + 1)  # 130: V tile stride (per head: 64 cols + ones col)


def build_kernel():
    nc = bacc.Bacc("TRN2", target_bir_lowering=False, debug=False)

    xT = nc.dram_tensor("xT", [D, BT], F32R, kind="ExternalInput")
    # wqkv packed on host as [128, KC, 3*CW]: (kc,:) = rows kc*128..+128 of
    # [w_q_slice | w_k_slice | w_v_slice]
    wqkv = nc.dram_tensor("wqkv", [128, KC * 3 * CW], F32R, kind="ExternalInput")
    wp = nc.dram_tensor("wp", [CW, D], F32R, kind="ExternalInput")
    out = nc.dram_tensor("out", [BT, D], F32, kind="ExternalOutput")

    with tile.TileContext(nc) as tc:
        _body(tc, xT.ap(), wqkv.ap(), wp.ap(), out.ap())
    nc.compile()
    return nc


def _body(tc, xT, wqkv, wp, out):
    nc = tc.nc
    with (
        tc.tile_pool(name="const", bufs=1) as const,
        tc.tile_pool(name="xin", bufs=2) as xin,
        tc.tile_pool(name="qk", bufs=2) as qkpool,
        tc.tile_pool(name="vb", bufs=2) as vbpool,
        tc.tile_pool(name="vs", bufs=2) as vspool,
        tc.tile_pool(name="pt", bufs=3) as ptpool,
        tc.tile_pool(name="ptd", bufs=2) as ptdpool,
        tc.tile_pool(name="yt", bufs=2) as ytpool,
        tc.tile_pool(name="dn", bufs=2) as dnpool,
        tc.tile_pool(name="os", bufs=2) as ospool,
        tc.tile_pool(name="pst", bufs=2, space="PSUM") as pst,
        tc.tile_pool(name="pav", bufs=1, space="PSUM") as pav,
        tc.tile_pool(name="psm", bufs=2, space="PSUM") as psm,
    ):
        # ---- constants ----
        wq_sb = const.tile([128, KC, 3 * CW], F32R, tag="wqkv")
        nc.sync.dma_start(wq_sb[:], wqkv.rearrange("p (k c) -> p k c", k=KC))
        wp_sb = const.tile([CW, D], F32R, tag="wp")
        nc.sync.dma_start(wp_sb[:], wp[:])
        ident = const.tile([128, 128], F32, tag="ident")
        make_identity(nc, ident[:])
        ones32 = const.tile([128, (T // 128) * HPC], F32, tag="ones32")
        nc.gpsimd.memset(ones32[:], 1.0)
        scale = 1.0 / float(np.sqrt(HD))

        def qkv_proj(b, qt, kt, vb):
            tok0 = b * T
            for ch in range(T // NQ):
                xt = xin.tile([128, KC, NQ], F32R, tag="xt")
                for kc in range(KC):
                    nc.sync.dma_start(
                        xt[:, kc, :],
                        xT[kc * 128 : (kc + 1) * 128,
                           tok0 + ch * NQ : tok0 + (ch + 1) * NQ],
                    )
                # Q^T and K^T m-tiles
                for m, dst in ((0, qt), (1, kt)):
                    ps = psm.tile([128, NQ], F32, tag="ps")
                    for kc in range(KC):
                        nc.tensor.matmul(
                            ps[:],
                            wq_sb[:, kc, m * CW : (m + 1) * CW],
                            xt[:, kc, :],
                            start=(kc == 0),
                            stop=(kc == KC - 1),
                        )
                    nc.vector.tensor_copy(dst[:, ch * NQ : (ch + 1) * NQ], ps[:])
                # V^T m-tile, then transpose into token-major layout
                ps = psm.tile([128, NQ], F32, tag="ps")
                for kc in range(KC):
                    nc.tensor.matmul(
                        ps[:],
                        wq_sb[:, kc, 2 * CW : 3 * CW],
                        xt[:, kc, :],
                        start=(kc == 0),
                        stop=(kc == KC - 1),
                    )
                vs = vspool.tile([128, NQ], F32, tag="vs")
                nc.vector.tensor_copy(vs[:], ps[:])
                pt2 = psm.tile([128, NQ], F32, tag="ps")
                for q in range(NG):
                    nc.tensor.transpose(
                        pt2[:, q * 128 : (q + 1) * 128],
                        vs[:, q * 128 : (q + 1) * 128],
                        ident[:],
                    )
                # pt2 holds [tok 128][tile q: h0 64 | h1 64]; scatter into vb
                # (col 0 of each 65-col head block is the ones column)
                dstv = bass.AP(
                    vb.tensor,
                    vb[:].offset + ch * NG * VST,
                    [vb[:].ap[0], [VST, NG], [HD + 1, HPC], [1, HD]],
                )
                srcv = pt2[:].rearrange("p (t h d) -> p t h d", t=NG, h=HPC)
                nc.vector.tensor_copy(dstv, srcv)
            # ones columns (denominator trick): col 65*j + HD of vb
            onesv = bass.AP(
                vb.tensor,
                vb[:].offset + HD,
                [vb[:].ap[0], [HD + 1, (T // 128) * HPC]],
            )
            nc.vector.tensor_copy(onesv, ones32[:])

        def finalize_norm(yt, jq, ytu):
            # divide O^T rows by the denominator row (broadcast to 64 parts)
            q0 = jq * NQ
            dn = dnpool.tile([1, HPC * NQ], F32, tag="dn")
            nc.vector.reciprocal(dn[:], ytu[HD : HD + 1, :])
            dnb = dnpool.tile([HD, HPC * NQ], F32, tag="dnb")
            nc.gpsimd.partition_broadcast(dnb[:], dn[:])
            for h in range(HPC):
                nc.vector.tensor_mul(
                    yt[h * HD : (h + 1) * HD, q0 : q0 + NQ],
                    ytu[0:HD, h * NQ : (h + 1) * NQ],
                    dnb[:, h * NQ : (h + 1) * NQ],
                )

        def attention(b, qt, kt, vb, yt):
            # Both heads processed together per kk-tile: h0 lives in SBUF
            # partitions 0-63, h1 in 64-127, so the S^T matmul pairs land on
            # PE row-tiles (64,128)@(0,0) and @(64,0) and can overlap.
            pending = None
            for jq in range(T // NQ):
                q0 = jq * NQ
                av0 = pav.tile([128, NQ], F32, tag="av0")
                av1 = pav.tile([128, NQ], F32, tag="av1")
                avs = [av0, av1]
                nkk = NG * (jq + 1)
                diag0 = NG * jq
                for kk in range(nkk):
                    i = kk - diag0          # >= 0 on the diagonal run
                    c0 = max(i, 0) * 128    # first valid q col in this chunk
                    w = NQ - c0
                    st = pst.tile([128, HPC * NQ], F32, tag="st")
                    for h in range(HPC):
                        nc.tensor.matmul(
                            st[:, h * NQ + c0 : (h + 1) * NQ],
                            kt[h * HD : (h + 1) * HD, kk * 128 : (kk + 1) * 128],
                            qt[h * HD : (h + 1) * HD, q0 + c0 : q0 + NQ],
                            start=True,
                            stop=True,
                        )
                    ptk = ptpool.tile([128, HPC * NQ], F32R, tag="pt")
                    stv = bass.AP(st.tensor, st[:].offset + c0,
                                  [st[:].ap[0], [NQ, HPC], [1, w]])
                    ptv = bass.AP(ptk.tensor, ptk[:].offset + c0,
                                  [ptk[:].ap[0], [NQ, HPC], [1, w]])
                    nc.scalar.activation(ptv, stv, EXP, scale=scale)
                    if i >= 0:
                        # zero q < kpart inside the 128-wide diagonal block
                        tri = bass.AP(ptk.tensor, ptk[:].offset + c0,
                                      [ptk[:].ap[0], [NQ, HPC], [1, 128]])
                        nc.gpsimd.affine_select(
                            out=tri,
                            in_=tri,
                            pattern=[[0, HPC], [1, 128]],
                            channel_multiplier=-1,
                            base=0,
                            compare_op=mybir.AluOpType.is_ge,
                            fill=0.0,
                        )
                    for h in range(HPC):
                        nc.tensor.matmul(
                            avs[h][0 : HD + 1, c0:NQ],
                            vb[:, kk * VST + h * (HD + 1) :
                                 kk * VST + (h + 1) * (HD + 1)],
                            ptk[:, h * NQ + c0 : (h + 1) * NQ],
                            start=(kk == 0),
                            stop=(kk == nkk - 1),
                        )
                # evacuate PSUM promptly (frees the av slots); rows 0..63 are
                # the unnormalized O^T, row 64 the denominator
                ytu = dnpool.tile([HD + 1, HPC * NQ], F32, tag="ytu")
                for h in range(HPC):
                    nc.vector.tensor_copy(
                        ytu[:, h * NQ : (h + 1) * NQ], avs[h][0 : HD + 1, :]
                    )
                if pending is not None:
                    finalize_norm(yt, *pending)
                pending = (jq, ytu)
            finalize_norm(yt, *pending)

        def out_proj(b, yt):
            tok0 = b * T
            for tt in range(T // 128):
                os_ = ospool.tile([128, D], F32, tag="os")
                for nn in range(D // NQ):
                    pp = psm.tile([128, NQ], F32, tag="ps")
                    nc.tensor.matmul(
                        pp[:],
                        yt[:, tt * 128 : (tt + 1) * 128],
                        wp_sb[:, nn * NQ : (nn + 1) * NQ],
                        start=True,
                        stop=True,
                    )
                    nc.vector.tensor_copy(os_[:, nn * NQ : (nn + 1) * NQ], pp[:])
                nc.sync.dma_start(
                    out[tok0 + tt * 128 : tok0 + (tt + 1) * 128, :], os_[:]
                )

        # out_proj(b) is emitted after qkv_proj(b+1) so the PE always has
        # independent work while batch b's last normalization drains.
        prev = None
        for b in range(B):
            qt = qkpool.tile([128, T], F32R, tag="qt")
            kt = qkpool.tile([128, T], F32R, tag="kt")
            vb = vbpool.tile([128, (T // 128) * VST], F32R, tag="vb")
            yt = ytpool.tile([128, T], F32R, tag="yt")
            qkv_proj(b, qt, kt, vb)
            if prev is not None:
                out_proj(*prev)
            attention(b, qt, kt, vb, yt)
            prev = (b, yt)
        out_proj(*prev)


_NC_CACHE = None


def kernel(x: np.ndarray, w_attn: np.ndarray, w_proj: np.ndarray) -> np.ndarray:
    global _NC_CACHE
    if _NC_CACHE is None:
        _NC_CACHE = build_kernel()
    nc = _NC_CACHE

    x = np.asarray(x, dtype=np.float32)
    w_attn = np.asarray(w_attn, dtype=np.float32)
    w_proj = np.asarray(w_proj, dtype=np.float32)

    xT = np.ascontiguousarray(x.reshape(BT, D).T)  # [D, BT]

    in_maps = []
    for c in range(NCORES):
        c0 = c * CW
        wq = w_attn[:, c0 : c0 + CW]
        wk = w_attn[:, D + c0 : D + c0 + CW]
        wv = w_attn[:, 2 * D + c0 : 2 * D + c0 + CW]
        wslice = np.concatenate([wq, wk, wv], axis=1)          # [D, 3*CW]
        wpacked = np.ascontiguousarray(
            wslice.reshape(KC, 128, 3 * CW).transpose(1, 0, 2)
        ).reshape(128, KC * 3 * CW)
        wpc = np.ascontiguousarray(w_proj[c0 : c0 + CW, :])    # [CW, D]
        in_maps.append({"xT": xT, "wqkv": wpacked, "wp": wpc})

    res = run_bass_kernel_spmd(nc, in_maps, core_ids=list(range(NCORES)))
    acc = np.zeros((BT, D), dtype=np.float32)
    for r in res.results:
        acc += r["out"]
    return acc.reshape(B, T, D)


if __name__ == "__main__":
    inputs = {
        "x": np.random.randn(B, T, D).astype(np.float32),
        "w_attn": (np.random.randn(D, 3 * D) / np.sqrt(D)).astype(np.float32),
        "w_proj": (np.random.randn(D, D) / np.sqrt(D)).astype(np.float32),
    }
    y = kernel(**inputs)
    print(y.shape, y.dtype)



# revision 3
# speedup vs baseline: 1.4204x; 1.4204x over previous
"""Causal self-attention on 8 NeuronCores (TRN2), tensor-parallel over heads.

Reference: y = proj(softmax(causal(Q K^T / sqrt(64))) V) with
B=4, T=2048, D=1024, H=16 heads, head_dim=64.

Sharding: each core owns 2 heads (a 128-column slice of the Q/K/V
projections and the matching 128 rows of w_proj) for all batches. Each
core emits a partial [B*T, D] output (bf16); the host sums the 8
partials in fp32 (row-parallel matmul unshard) and reshapes to [B,T,D].

v2 design vs baseline:
  - all matmul operands bf16 (FWL fast weight loads, half DMA traffic)
  - weight-stationary Q/K projection (2 PSUM banks, LDW amortized)
  - V projected directly token-major (x tiles stationary, w_v moving)
    so no PE transposes are needed for the AV lhsT
  - softmax normalization deferred: unnormalized O and the denominator
    row come out of the AV matmuls (ones column trick); 1/den via
    reciprocal_approx_fast, partition-broadcast, and the divide is
    fused into the PSUM->SBUF evacuation multiply
  - out-proj evacuation split across DVE and ACT, output DMA'd as bf16
"""

import sys

for _p in ("/opt/trn_rl_repo",):
    if _p not in sys.path:
        sys.path.insert(0, _p)

import ml_dtypes
import numpy as np

import concourse.bass as bass
import concourse.bacc as bacc
import concourse.mybir as mybir
from concourse import tile
from concourse.bass_utils import run_bass_kernel_spmd

B, T, D, H = 4, 2048, 1024, 16
HD = D // H           # 64 head dim
NCORES = 8
HPC = H // NCORES     # 2 heads per core
CW = HPC * HD         # 128: per-core qkv column slice width
BT = B * T            # 8192 tokens
KC = D // 128         # 8 contraction chunks for the qkv projection
NQ = 512              # query chunk
NG = NQ // 128        # 4 key-tiles per S^T group
F32 = mybir.dt.float32
BF16 = mybir.dt.bfloat16
EXP = mybir.ActivationFunctionType.Exp
BF = ml_dtypes.bfloat16

VST = HPC * (HD + 1)  # 130: V tile stride (per head: 64 cols + ones col)
NKK = T // 128        # 16 key tiles per batch
VBW = NKK * VST + 64  # vb width incl. tail pad for the 128-wide AV ldweights


def build_kernel():
    nc = bacc.Bacc("TRN2", target_bir_lowering=False, debug=False)

    xT = nc.dram_tensor("xT", [D, BT], BF16, kind="ExternalInput")
    # wqkv packed on host as [128, KC, 3*CW]: (kc,:) = rows kc*128..+128 of
    # [w_q_slice | w_k_slice | w_v_slice]
    wqkv = nc.dram_tensor("wqkv", [128, KC * 3 * CW], BF16, kind="ExternalInput")
    wp = nc.dram_tensor("wp", [CW, D], BF16, kind="ExternalInput")
    out = nc.dram_tensor("out", [BT, D], BF16, kind="ExternalOutput")

    with tile.TileContext(nc) as tc:
        _body(tc, xT.ap(), wqkv.ap(), wp.ap(), out.ap())
    nc.compile()
    return nc


def _body(tc, xT, wqkv, wp, out):
    nc = tc.nc
    with (
        tc.tile_pool(name="const", bufs=1) as const,
        tc.tile_pool(name="xin", bufs=2) as xin,
        tc.tile_pool(name="qk", bufs=2) as qkpool,
        tc.tile_pool(name="vb", bufs=2) as vbpool,
        tc.tile_pool(name="pt", bufs=3) as ptpool,
        tc.tile_pool(name="yt", bufs=2) as ytpool,
        tc.tile_pool(name="dn", bufs=2) as dnpool,
        tc.tile_pool(name="os", bufs=3) as ospool,
        tc.tile_pool(name="psA", bufs=2, space="PSUM") as psA,
        tc.tile_pool(name="pst", bufs=2, space="PSUM") as pst,
        tc.tile_pool(name="pav", bufs=1, space="PSUM") as pav,
    ):
        # ---- constants ----
        wq_sb = const.tile([128, KC, 3 * CW], BF16, tag="wqkv")
        nc.sync.dma_start(wq_sb[:], wqkv.rearrange("p (k c) -> p k c", k=KC))
        wp_sb = const.tile([CW, D], BF16, tag="wp")
        nc.sync.dma_start(wp_sb[:], wp[:])
        ones32 = const.tile([128, NKK * HPC], BF16, tag="ones32")
        nc.gpsimd.memset(ones32[:], 1.0)
        scale = 1.0 / float(np.sqrt(HD))

        def qkv_proj(b, qt, kt, vb):
            tok0 = b * T
            xt = xin.tile([128, KC, T], BF16, tag="xt")
            for kc in range(KC):
                nc.sync.dma_start(
                    xt[:, kc, :],
                    xT[kc * 128 : (kc + 1) * 128, tok0 : tok0 + T],
                )
            # Q^T and K^T, weight-stationary over kc, 2 PSUM banks
            for m, dst in ((0, qt), (1, kt)):
                for g in range(2):
                    ps0 = psA.tile([128, NQ], F32, tag="ps")
                    ps1 = psA.tile([128, NQ], F32, tag="ps")
                    t0 = g * 2 * NQ
                    for kc in range(KC):
                        w_ap = wq_sb[:, kc, m * CW : (m + 1) * CW]
                        nc.tensor.matmul(
                            ps0[:], w_ap, xt[:, kc, t0 : t0 + NQ],
                            start=(kc == 0), stop=(kc == KC - 1),
                        )
                        nc.tensor.matmul(
                            ps1[:], w_ap, xt[:, kc, t0 + NQ : t0 + 2 * NQ],
                            start=(kc == 0), stop=(kc == KC - 1),
                        )
                    nc.vector.tensor_copy(dst[:, t0 : t0 + NQ], ps0[:])
                    nc.vector.tensor_copy(dst[:, t0 + NQ : t0 + 2 * NQ], ps1[:])
            # V^T token-major: x tiles stationary, w_v moving; out [tok, c]
            for g in range(NKK // 4):
                psv = psA.tile([128, NQ], F32, tag="ps")
                for tt in range(4):
                    kk = g * 4 + tt
                    for kc in range(KC):
                        nc.tensor.matmul(
                            psv[:, tt * 128 : (tt + 1) * 128],
                            xt[:, kc, kk * 128 : (kk + 1) * 128],
                            wq_sb[:, kc, 2 * CW : 3 * CW],
                            start=(kc == 0), stop=(kc == KC - 1),
                        )
                # scatter 4 token-tiles into vb's 65-stride head blocks
                dstv = bass.AP(
                    vb.tensor,
                    vb[:].offset + g * 4 * VST,
                    [vb[:].ap[0], [VST, 4], [HD + 1, HPC], [1, HD]],
                )
                srcv = psv[:].rearrange("p (t h d) -> p t h d", t=4, h=HPC)
                nc.vector.tensor_copy(dstv, srcv)
            # ones columns (denominator trick): col 65*j + HD of each block
            onesv = bass.AP(
                vb.tensor,
                vb[:].offset + HD,
                [vb[:].ap[0], [HD + 1, NKK * HPC]],
            )
            nc.vector.tensor_copy(onesv, ones32[:])
            # tail pad so the 128-wide AV ldweights never reads junk
            padv = bass.AP(
                vb.tensor, vb[:].offset + NKK * VST, [vb[:].ap[0], [1, 64]]
            )
            nc.gpsimd.memset(padv, 0.0)

        def attention(b, qt, kt, vb, yt):
            for jq in range(T // NQ):
                q0 = jq * NQ
                av0 = pav.tile([128, NQ], F32, tag="av0")
                av1 = pav.tile([128, NQ], F32, tag="av1")
                avs = (av0, av1)
                nkk = NG * (jq + 1)
                diag0 = NG * jq
                for kk in range(nkk):
                    i = kk - diag0          # >= 0 on the diagonal run
                    c0 = max(i, 0) * 128    # first valid q col in this chunk
                    w = NQ - c0
                    st = pst.tile([128, HPC * NQ], F32, tag="st")
                    for h in range(HPC):
                        nc.tensor.matmul(
                            st[:, h * NQ + c0 : (h + 1) * NQ],
                            kt[h * HD : (h + 1) * HD, kk * 128 : (kk + 1) * 128],
                            qt[h * HD : (h + 1) * HD, q0 + c0 : q0 + NQ],
                            start=True,
                            stop=True,
                        )
                    ptk = ptpool.tile([128, HPC * NQ], BF16, tag="pt")
                    stv = bass.AP(st.tensor, st[:].offset + c0,
                                  [st[:].ap[0], [NQ, HPC], [1, w]])
                    ptv = bass.AP(ptk.tensor, ptk[:].offset + c0,
                                  [ptk[:].ap[0], [NQ, HPC], [1, w]])
                    nc.scalar.activation(ptv, stv, EXP, scale=scale)
                    if i >= 0:
                        # zero q < kpart inside the 128-wide diagonal block
                        tri = bass.AP(ptk.tensor, ptk[:].offset + c0,
                                      [ptk[:].ap[0], [NQ, HPC], [1, 128]])
                        nc.gpsimd.affine_select(
                            out=tri,
                            in_=tri,
                            pattern=[[0, HPC], [1, 128]],
                            channel_multiplier=-1,
                            base=0,
                            compare_op=mybir.AluOpType.is_ge,
                            fill=0.0,
                        )
                    for h in range(HPC):
                        # 128-wide stationary: head h's 65 cols + 63 junk
                        nc.tensor.matmul(
                            avs[h][:, c0:NQ],
                            vb[:, kk * VST + h * (HD + 1) :
                                 kk * VST + h * (HD + 1) + 128],
                            ptk[:, h * NQ + c0 : (h + 1) * NQ],
                            start=(kk == 0),
                            stop=(kk == nkk - 1),
                        )
                # evacuate: rows 0..63 unnormalized O^T, row 64 denominator.
                # y = O * (1/den) fused into the evacuation multiply.
                for h in range(HPC):
                    # custom DVE ops don't honor a nonzero base partition on
                    # the input AP: stage the den row to partition 0 first
                    dnr = dnpool.tile([1, NQ], F32, tag=f"d{h}")
                    nc.vector.tensor_copy(dnr[:], avs[h][HD : HD + 1, :])
                    r = dnpool.tile([1, NQ], F32, tag=f"r{h}")
                    nc.vector.reciprocal_approx_fast(r[:], dnr[:])
                    R = dnpool.tile([HD, NQ], F32, tag=f"R{h}")
                    nc.gpsimd.partition_broadcast(R[:], r[:])
                    nc.vector.tensor_mul(
                        yt[h * HD : (h + 1) * HD, q0 : q0 + NQ],
                        avs[h][0:HD, :],
                        R[:],
                    )

        def out_proj(b, yt):
            tok0 = b * T
            for tt in range(T // 128):
                os_ = ospool.tile([128, D], BF16, tag="os")
                for nn in range(D // NQ):
                    pp = psA.tile([128, NQ], F32, tag="ps")
                    nc.tensor.matmul(
                        pp[:],
                        yt[:, tt * 128 : (tt + 1) * 128],
                        wp_sb[:, nn * NQ : (nn + 1) * NQ],
                        start=True,
                        stop=True,
                    )
                    # split evacuation across DVE and ACT
                    if nn == 0:
                        nc.vector.tensor_copy(os_[:, nn * NQ : (nn + 1) * NQ], pp[:])
                    else:
                        nc.scalar.copy(os_[:, nn * NQ : (nn + 1) * NQ], pp[:])
                nc.sync.dma_start(
                    out[tok0 + tt * 128 : tok0 + (tt + 1) * 128, :], os_[:]
                )

        # out_proj(b) is emitted after qkv_proj(b+1) so the PE always has
        # independent work while batch b's attention drains.
        prev = None
        for b in range(B):
            qt = qkpool.tile([128, T], BF16, tag="qt")
            kt = qkpool.tile([128, T], BF16, tag="kt")
            vb = vbpool.tile([128, VBW], BF16, tag="vb")
            yt = ytpool.tile([128, T], BF16, tag="yt")
            qkv_proj(b, qt, kt, vb)
            if prev is not None:
                out_proj(*prev)
            attention(b, qt, kt, vb, yt)
            prev = (b, yt)
        out_proj(*prev)


_NC_CACHE = None


def make_in_maps(x, w_attn, w_proj):
    x = np.asarray(x, dtype=np.float32)
    w_attn = np.asarray(w_attn, dtype=np.float32)
    w_proj = np.asarray(w_proj, dtype=np.float32)

    xT = np.ascontiguousarray(x.reshape(BT, D).T).astype(BF)  # [D, BT]

    in_maps = []
    for c in range(NCORES):
        c0 = c * CW
        wq = w_attn[:, c0 : c0 + CW]
        wk = w_attn[:, D + c0 : D + c0 + CW]
        wv = w_attn[:, 2 * D + c0 : 2 * D + c0 + CW]
        wslice = np.concatenate([wq, wk, wv], axis=1)          # [D, 3*CW]
        wpacked = np.ascontiguousarray(
            wslice.reshape(KC, 128, 3 * CW).transpose(1, 0, 2)
        ).reshape(128, KC * 3 * CW).astype(BF)
        wpc = np.ascontiguousarray(w_proj[c0 : c0 + CW, :]).astype(BF)
        in_maps.append({"xT": xT, "wqkv": wpacked, "wp": wpc})
    return in_maps


def kernel(x: np.ndarray, w_attn: np.ndarray, w_proj: np.ndarray) -> np.ndarray:
    global _NC_CACHE
    if _NC_CACHE is None:
        _NC_CACHE = build_kernel()
    nc = _NC_CACHE

    in_maps = make_in_maps(x, w_attn, w_proj)
    res = run_bass_kernel_spmd(nc, in_maps, core_ids=list(range(NCORES)))
    acc = np.zeros((BT, D), dtype=np.float32)
    for r in res.results:
        acc += np.asarray(r["out"], dtype=np.float32)
    return acc.reshape(B, T, D)


if __name__ == "__main__":
    inputs = {
        "x": np.random.randn(B, T, D).astype(np.float32),
        "w_attn": (np.random.randn(D, 3 * D) / np.sqrt(D)).astype(np.float32),
        "w_proj": (np.random.randn(D, D) / np.sqrt(D)).astype(np.float32),
    }
    y = kernel(**inputs)
    print(y.shape, y.dtype)
